# revision 1
# baseline (speedup 1.0000x reference)
"""Trainium2 Bass kernel for GrapherModule (fc1+BN -> KNN(k=9) -> MaxRelative conv+BN+GELU -> fc2+BN -> +residual).

Sharding: 8 cores; core d handles batch b=d//4, query slice qoff=(d%4)*2048.
Each core receives its batch's nodes np.roll'ed by -qoff so that its queries are
always local nodes 0..2047 -> the SPMD program is identical on every core.
BN statistics are global over B*N tokens: BN1 stats computed replicated
(both batches on every core); BNc / BN2 stats via tiny AllReduce collectives.
"""
import sys, os
sys.path.insert(0, '/opt/trn_rl_repo')
os.environ.setdefault('JAX_PLATFORMS', 'cpu')

import numpy as np

B, N, C = 2, 8192, 128
K = 9
NQ = 2048          # queries per core
NT = NQ // 128     # 16 query tiles per core
EPS = 1e-5

_CACHE = {}


def _build():
    import concourse.bass as bass
    import concourse.mybir as mybir
    import concourse.tile as tile
    from concourse import bacc
    from concourse.masks import make_identity

    dt = mybir.dt
    AF = mybir.ActivationFunctionType
    ALU = mybir.AluOpType
    AX = mybir.AxisListType

    nc = bacc.Bacc("TRN2", target_bir_lowering=False, debug=False,
                   enable_asserts=False, num_devices=8)

    # ---- I/O ----
    x_own = nc.dram_tensor("x_own", [N, C], dt.float32, kind="ExternalInput")
    x_oth = nc.dram_tensor("x_oth", [N, C], dt.float32, kind="ExternalInput")
    fc1_w = nc.dram_tensor("fc1_w", [C, C], dt.float32, kind="ExternalInput")
    fc1_b = nc.dram_tensor("fc1_b", [C], dt.float32, kind="ExternalInput")
    bn1_g = nc.dram_tensor("bn1_g", [C], dt.float32, kind="ExternalInput")
    bn1_b = nc.dram_tensor("bn1_b", [C], dt.float32, kind="ExternalInput")
    conv_w = nc.dram_tensor("conv_w", [C, 2 * C], dt.float32, kind="ExternalInput")
    conv_b = nc.dram_tensor("conv_b", [C], dt.float32, kind="ExternalInput")
    bnc_g = nc.dram_tensor("bnc_g", [C], dt.float32, kind="ExternalInput")
    bnc_b = nc.dram_tensor("bnc_b", [C], dt.float32, kind="ExternalInput")
    fc2_w = nc.dram_tensor("fc2_w", [C, C], dt.float32, kind="ExternalInput")
    fc2_b = nc.dram_tensor("fc2_b", [C], dt.float32, kind="ExternalInput")
    bn2_g = nc.dram_tensor("bn2_g", [C], dt.float32, kind="ExternalInput")
    bn2_b = nc.dram_tensor("bn2_b", [C], dt.float32, kind="ExternalInput")
    y = nc.dram_tensor("y", [NQ, C], dt.float32, kind="ExternalOutput")
    idx_out = nc.dram_tensor("idx_out", [NQ, 9], dt.uint32, kind="ExternalOutput")
    h_fm_out = nc.dram_tensor("h_fm_out", [C, N], dt.float32, kind="ExternalOutput")
    h_tok_out = nc.dram_tensor("h_tok_out", [N, C], dt.float32, kind="ExternalOutput")
    DBG = os.environ.get('KDBG') == '1'
    if DBG:
        dbg_h = nc.dram_tensor("dbg_h", [C, N], dt.float32, kind="ExternalOutput")
        dbg_nx = nc.dram_tensor("dbg_nx", [C, N], dt.float32, kind="ExternalOutput")
        dbg_st = nc.dram_tensor("dbg_st", [C, 8], dt.float32, kind="ExternalOutput")
        dbg_s = nc.dram_tensor("dbg_s", [C, N], dt.float32, kind="ExternalOutput")
        dbg_i = nc.dram_tensor("dbg_i", [C, 9], dt.uint32, kind="ExternalOutput")
        dbg_cp = nc.dram_tensor("dbg_cp", [C, NQ], dt.float32, kind="ExternalOutput")
        dbg_gt = nc.dram_tensor("dbg_gt", [C, 9 * 128], dt.float32, kind="ExternalOutput")
        dbg_mx = nc.dram_tensor("dbg_mx", [C, 128], dt.float32, kind="ExternalOutput")
        dbg_ht = nc.dram_tensor("dbg_ht", [128, C], dt.float32, kind="ExternalOutput")
    h_tok = nc.dram_tensor("h_tok", [N, C], dt.float32, kind="Internal")

    def col(t):  # [C] dram -> [C,1] view
        return t[:].rearrange("(c one) -> c one", one=1)

    def rowT(t, r0, r1):  # dram [M, Kc] -> transposed view [Kc, r1-r0]
        return t[r0:r1, :].rearrange("m k -> k m")

    with tile.TileContext(nc) as tc:
        wpool = tc.alloc_tile_pool(name="w", bufs=1)
        pers = tc.alloc_tile_pool(name="pers", bufs=1)
        psA = tc.alloc_tile_pool(name="psA", bufs=4, space="PSUM")
        psB = tc.alloc_tile_pool(name="psB", bufs=4, space="PSUM")
        dram = tc.alloc_tile_pool(name="dram", bufs=2, space="DRAM")

        ident = wpool.tile([128, 128], dt.float32)
        make_identity(nc, ident[:])
        ones128 = wpool.tile([128, 128], dt.float32)
        nc.vector.memset(ones128[:], 1.0)

        # weights (transposed views loaded via strided DMA once)
        fc1wT = wpool.tile([C, C], dt.float32)
        nc.sync.dma_start(fc1wT[:], rowT(fc1_w, 0, C))
        cw1T = wpool.tile([C, C], dt.float32)
        cw2T = wpool.tile([C, C], dt.float32)
        nc.sync.dma_start(cw1T[:], conv_w[:, 0:C].rearrange("m k -> k m"))
        nc.sync.dma_start(cw2T[:], conv_w[:, C:2 * C].rearrange("m k -> k m"))
        fc2wT = wpool.tile([C, C], dt.float32)
        nc.sync.dma_start(fc2wT[:], rowT(fc2_w, 0, C))
        fc1b = wpool.tile([C, 1], dt.float32); nc.sync.dma_start(fc1b[:], col(fc1_b))
        bn1g = wpool.tile([C, 1], dt.float32); nc.sync.dma_start(bn1g[:], col(bn1_g))
        bn1bb = wpool.tile([C, 1], dt.float32); nc.sync.dma_start(bn1bb[:], col(bn1_b))
        convb = wpool.tile([C, 1], dt.float32); nc.sync.dma_start(convb[:], col(conv_b))
        bncg = wpool.tile([C, 1], dt.float32); nc.sync.dma_start(bncg[:], col(bnc_g))
        bncb = wpool.tile([C, 1], dt.float32); nc.sync.dma_start(bncb[:], col(bnc_b))
        fc2b = wpool.tile([C, 1], dt.float32); nc.sync.dma_start(fc2b[:], col(fc2_b))
        bn2g = wpool.tile([C, 1], dt.float32); nc.sync.dma_start(bn2g[:], col(bn2_g))
        bn2bb = wpool.tile([C, 1], dt.float32); nc.sync.dma_start(bn2bb[:], col(bn2_b))

        # ---------- Phase A: fc1 + BN1 stats over both batches ----------
        h = pers.tile([128, N], dt.float32)          # own-batch pre-h then h (in-place norm)
        sum_p = pers.tile([128, 128], dt.float32)
        ssq_p = pers.tile([128, 128], dt.float32)

        with tc.tile_pool(name="phA", bufs=4) as phA, \
             tc.tile_pool(name="phAj", bufs=4) as phAj:
            for t in range(128):
                own = t < 64
                src = x_own if own else x_oth
                r0 = (t % 64) * 128
                xt = phA.tile([128, 128], dt.float32, tag="xt")
                nc.sync.dma_start(xt[:], src[r0:r0 + 128, :])
                pxt = psB.tile([128, 128], dt.float32, tag="p128")
                nc.tensor.transpose(pxt[:], xt[:], ident[:])
                xT = phA.tile([128, 128], dt.float32, tag="xT")
                nc.vector.tensor_copy(xT[:], pxt[:])
                pre = psB.tile([128, 128], dt.float32, tag="p128")
                nc.tensor.matmul(pre[:], fc1wT[:], xT[:], start=True, stop=True)
                dest = h[:, r0:r0 + 128] if own else phAj.tile([128, 128], dt.float32, tag="jd")
                nc.scalar.activation(dest if own else dest[:], pre[:], AF.Identity,
                                     bias=fc1b[:], accum_out=sum_p[:, t:t + 1])
                junk = phAj.tile([128, 128], dt.float32, tag="jq")
                nc.scalar.activation(junk[:], dest if own else dest[:], AF.Square,
                                     accum_out=ssq_p[:, t:t + 1])

        # BN1 scale/bias
        stat = pers.tile([128, 8], dt.float32)
        mean1, ex2, var1, rstd1, sc1, bi1 = (stat[:, i:i + 1] for i in range(6))
        nc.vector.reduce_sum(mean1, sum_p[:], axis=AX.X)
        nc.vector.tensor_scalar_mul(mean1, mean1, 1.0 / (B * N))
        nc.vector.reduce_sum(ex2, ssq_p[:], axis=AX.X)
        nc.vector.tensor_scalar_mul(ex2, ex2, 1.0 / (B * N))
        nc.vector.tensor_tensor(var1, mean1, mean1, op=ALU.mult)
        nc.vector.tensor_sub(var1, ex2, var1)
        nc.vector.tensor_scalar(var1, var1, EPS, None, op0=ALU.add)
        nc.vector.reciprocal(rstd1, var1)
        nc.scalar.activation(rstd1, rstd1, AF.Sqrt)
        nc.vector.tensor_tensor(sc1, rstd1, bn1g[:], op=ALU.mult)
        nc.vector.tensor_tensor(bi1, mean1, sc1, op=ALU.mult)
        nc.vector.tensor_sub(bi1, bn1bb[:], bi1)
        # normalize own batch in place: h = pre*sc1 + bi1
        nc.scalar.activation(h[:], h[:], AF.Identity, bias=bi1, scale=sc1)

        if DBG:
            nc.sync.dma_start(dbg_h[:], h[:])
            nc.sync.dma_start(dbg_st[:], stat[:])
        # negx2[128, N] = -0.5 * colsum(h*h), broadcast via ones-matmul
        negx2 = pers.tile([128, N], dt.float32)
        with tc.tile_pool(name="hh", bufs=2) as hhp:
            for c in range(16):
                sl = slice(c * 512, (c + 1) * 512)
                hh = hhp.tile([128, 512], dt.float32, tag="hh")
                nc.vector.tensor_tensor(hh[:], h[:, sl], h[:, sl], op=ALU.mult)
                pn = psA.tile([128, 512], dt.float32, tag="ps")
                nc.tensor.matmul(pn[:], ones128[:], hh[:], start=True, stop=True)
                nc.scalar.activation(negx2[:, sl], pn[:], AF.Copy, scale=-0.5)

        if DBG:
            nc.sync.dma_start(dbg_nx[:], negx2[:])
        # h_tok (token-major h) to DRAM for the gather
        with tc.tile_pool(name="ht", bufs=4) as htp:
            for i in range(64):
                r0 = i * 128
                pt = psB.tile([128, 128], dt.float32, tag="p128")
                nc.tensor.transpose(pt[:], h[:, r0:r0 + 128], ident[:])
                st = htp.tile([128, 128], dt.float32, tag="st")
                nc.vector.tensor_copy(st[:], pt[:])
                nc.sync.dma_start(h_tok_out[r0:r0 + 128, :], st[:])

        # ---------- Phase B: selection + gather + conv ----------
        with tc.tile_pool(name="sel", bufs=2) as selp, \
             tc.tile_pool(name="smal", bufs=4) as smal, \
             tc.tile_pool(name="gath", bufs=3) as gthp:
            for i in range(NT):
                q0 = i * 128
                s = selp.tile([128, N], dt.float32, tag="s")
                for c in range(16):
                    sl = slice(c * 512, (c + 1) * 512)
                    ps = psA.tile([128, 512], dt.float32, tag="ps")
                    nc.tensor.matmul(ps[:], h[:, q0:q0 + 128], h[:, sl],
                                     start=True, stop=True)
                    nc.vector.scalar_tensor_tensor(
                        s[:, sl], ps[:], 1.0, negx2[:, sl],
                        op0=ALU.mult, op1=ALU.add)
                # mask self (diagonal of the query block)
                nc.gpsimd.affine_select(
                    s[:, q0:q0 + 128], s[:, q0:q0 + 128],
                    pattern=[[1, 128]], compare_op=ALU.not_equal,
                    fill=-1e30, base=0, channel_multiplier=-1)
                top8v = smal.tile([128, 8], dt.float32, tag="t8v")
                nc.vector.max(top8v[:], s[:])
                idx9 = smal.tile([128, 9], dt.uint32, tag="i9")
                nc.gpsimd.iota(idx9[:, 0:1], pattern=[[0, 1]], base=q0,
                               channel_multiplier=1)
                nc.vector.max_index(idx9[:, 1:9], top8v[:], s[:])
                if DBG and i == 0:
                    nc.sync.dma_start(dbg_s[:], s[:])
                    nc.sync.dma_start(dbg_i[:], idx9[:])
                nc.sync.dma_start(idx_out[q0:q0 + 128, :], idx9[:])

        nc.sync.dma_start(h_fm_out[:], h[:])

        for p in (dram, psB, psA, pers, wpool):
            p.release()

    nc.compile()
    return nc




def _build2():
    import concourse.bass as bass
    import concourse.mybir as mybir
    import concourse.tile as tile
    from concourse import bacc
    from concourse.masks import make_identity

    dt = mybir.dt
    AF = mybir.ActivationFunctionType
    ALU = mybir.AluOpType
    AX = mybir.AxisListType

    nc = bacc.Bacc("TRN2", target_bir_lowering=False, debug=False,
                   enable_asserts=False, num_devices=8)
    h_fm = nc.dram_tensor("h_fm", [C, N], dt.float32, kind="ExternalInput")
    maxn_fm = nc.dram_tensor("maxn_fm", [C, NQ], dt.float32, kind="ExternalInput")
    x_res = nc.dram_tensor("x_res", [NQ, C], dt.float32, kind="ExternalInput")
    conv_w = nc.dram_tensor("conv_w", [C, 2 * C], dt.float32, kind="ExternalInput")
    conv_b = nc.dram_tensor("conv_b", [C], dt.float32, kind="ExternalInput")
    bnc_g = nc.dram_tensor("bnc_g", [C], dt.float32, kind="ExternalInput")
    bnc_b = nc.dram_tensor("bnc_b", [C], dt.float32, kind="ExternalInput")
    fc2_w = nc.dram_tensor("fc2_w", [C, C], dt.float32, kind="ExternalInput")
    fc2_b = nc.dram_tensor("fc2_b", [C], dt.float32, kind="ExternalInput")
    bn2_g = nc.dram_tensor("bn2_g", [C], dt.float32, kind="ExternalInput")
    bn2_b = nc.dram_tensor("bn2_b", [C], dt.float32, kind="ExternalInput")
    y = nc.dram_tensor("y", [NQ, C], dt.float32, kind="ExternalOutput")

    def col(t):
        return t[:].rearrange("(c one) -> c one", one=1)

    with tile.TileContext(nc) as tc:
        wpool = tc.alloc_tile_pool(name="w", bufs=1)
        pers = tc.alloc_tile_pool(name="pers", bufs=1)
        psA = tc.alloc_tile_pool(name="psA", bufs=4, space="PSUM")
        psB = tc.alloc_tile_pool(name="psB", bufs=4, space="PSUM")
        dram = tc.alloc_tile_pool(name="dram", bufs=2, space="DRAM")

        ident = wpool.tile([128, 128], dt.float32)
        make_identity(nc, ident[:])
        cw1T = wpool.tile([C, C], dt.float32)
        cw2T = wpool.tile([C, C], dt.float32)
        nc.sync.dma_start(cw1T[:], conv_w[:, 0:C].rearrange("m k -> k m"))
        nc.sync.dma_start(cw2T[:], conv_w[:, C:2 * C].rearrange("m k -> k m"))
        fc2wT = wpool.tile([C, C], dt.float32)
        nc.sync.dma_start(fc2wT[:], fc2_w[:].rearrange("m k -> k m"))
        convb = wpool.tile([C, 1], dt.float32); nc.sync.dma_start(convb[:], col(conv_b))
        bncg = wpool.tile([C, 1], dt.float32); nc.sync.dma_start(bncg[:], col(bnc_g))
        bncb = wpool.tile([C, 1], dt.float32); nc.sync.dma_start(bncb[:], col(bnc_b))
        fc2b = wpool.tile([C, 1], dt.float32); nc.sync.dma_start(fc2b[:], col(fc2_b))
        bn2g = wpool.tile([C, 1], dt.float32); nc.sync.dma_start(bn2g[:], col(bn2_g))
        bn2bb = wpool.tile([C, 1], dt.float32); nc.sync.dma_start(bn2bb[:], col(bn2_b))

        hq = pers.tile([128, NQ], dt.float32)
        nc.sync.dma_start(hq[:], h_fm[:, 0:NQ])
        mxf = pers.tile([128, NQ], dt.float32)
        nc.sync.dma_start(mxf[:], maxn_fm[:])

        convpre = pers.tile([128, NQ], dt.float32)
        csum_p = pers.tile([128, 4], dt.float32)
        cssq_p = pers.tile([128, 4], dt.float32)
        with tc.tile_pool(name="cj", bufs=2) as cj:
            for c in range(4):
                sl = slice(c * 512, (c + 1) * 512)
                r2 = cj.tile([128, 512], dt.float32, tag="r2")
                nc.vector.tensor_sub(r2[:], mxf[:, sl], hq[:, sl])
                pc = psA.tile([128, 512], dt.float32, tag="ps")
                nc.tensor.matmul(pc[:], cw1T[:], hq[:, sl], start=True, stop=False)
                nc.tensor.matmul(pc[:], cw2T[:], r2[:], start=False, stop=True)
                nc.scalar.activation(convpre[:, sl], pc[:], AF.Identity,
                                     bias=convb[:], accum_out=csum_p[:, c:c + 1])
                jq = cj.tile([128, 512], dt.float32, tag="jq")
                nc.scalar.activation(jq[:], convpre[:, sl], AF.Square,
                                     accum_out=cssq_p[:, c:c + 1])

        def allreduce2(sump, ssqp):
            loc = pers.tile([128, 2], dt.float32)
            nc.vector.reduce_sum(loc[:, 0:1], sump[:], axis=AX.X)
            nc.vector.reduce_sum(loc[:, 1:2], ssqp[:], axis=AX.X)
            bin_ = dram.tile([128, 2], dt.float32)
            bout = dram.tile([128, 2], dt.float32)
            nc.gpsimd.dma_start(bin_[:], loc[:])
            nc.gpsimd.collective_compute(
                "AllReduce", ALU.add, replica_groups=[list(range(8))],
                ins=[bin_.opt()], outs=[bout.opt()])
            tot = pers.tile([128, 2], dt.float32)
            nc.gpsimd.dma_start(tot[:], bout[:])
            return tot

        def bnparams(tot, gam, bet):
            st = pers.tile([128, 8], dt.float32)
            mm, e2, vv, rr, sc, bi = (st[:, i:i + 1] for i in range(6))
            nc.vector.tensor_scalar_mul(mm, tot[:, 0:1], 1.0 / (B * N))
            nc.vector.tensor_scalar_mul(e2, tot[:, 1:2], 1.0 / (B * N))
            nc.vector.tensor_tensor(vv, mm, mm, op=ALU.mult)
            nc.vector.tensor_sub(vv, e2, vv)
            nc.vector.tensor_scalar(vv, vv, EPS, None, op0=ALU.add)
            nc.vector.reciprocal(rr, vv)
            nc.scalar.activation(rr, rr, AF.Sqrt)
            nc.vector.tensor_tensor(sc, rr, gam, op=ALU.mult)
            nc.vector.tensor_tensor(bi, mm, sc, op=ALU.mult)
            nc.vector.tensor_sub(bi, bet, bi)
            return sc, bi

        scc, bic = bnparams(allreduce2(csum_p, cssq_p), bncg[:], bncb[:])
        g = pers.tile([128, NQ], dt.float32)
        nc.scalar.activation(g[:], convpre[:], AF.Gelu, bias=bic, scale=scc)

        f2pre = pers.tile([128, NQ], dt.float32)
        fsum_p = pers.tile([128, 4], dt.float32)
        fssq_p = pers.tile([128, 4], dt.float32)
        with tc.tile_pool(name="fj", bufs=2) as fj:
            for c in range(4):
                sl = slice(c * 512, (c + 1) * 512)
                pf = psA.tile([128, 512], dt.float32, tag="ps")
                nc.tensor.matmul(pf[:], fc2wT[:], g[:, sl], start=True, stop=True)
                nc.scalar.activation(f2pre[:, sl], pf[:], AF.Identity, bias=fc2b[:],
                                     accum_out=fsum_p[:, c:c + 1])
                jf = fj.tile([128, 512], dt.float32, tag="jf")
                nc.scalar.activation(jf[:], f2pre[:, sl], AF.Square,
                                     accum_out=fssq_p[:, c:c + 1])

        scf, bif = bnparams(allreduce2(fsum_p, fssq_p), bn2g[:], bn2bb[:])
        outfm = pers.tile([128, NQ], dt.float32)
        nc.scalar.activation(outfm[:], f2pre[:], AF.Identity, bias=bif, scale=scf)

        with tc.tile_pool(name="op", bufs=4) as op:
            for i in range(NT):
                q0 = i * 128
                po = psB.tile([128, 128], dt.float32, tag="p128")
                nc.tensor.transpose(po[:], outfm[:, q0:q0 + 128], ident[:])
                xr = op.tile([128, 128], dt.float32, tag="xr")
                nc.sync.dma_start(xr[:], x_res[q0:q0 + 128, :])
                ot = op.tile([128, 128], dt.float32, tag="ot")
                nc.vector.tensor_add(ot[:], po[:], xr[:])
                nc.sync.dma_start(y[q0:q0 + 128, :], ot[:])

        for p in (dram, psB, psA, pers, wpool):
            p.release()

    nc.compile()
    return nc


def kernel(**inputs):
    from concourse import bass_utils

    if 'nc1' not in _CACHE:
        _CACHE['nc1'] = _build()
    if 'nc2' not in _CACHE:
        _CACHE['nc2'] = _build2()
    nc1, nc2 = _CACHE['nc1'], _CACHE['nc2']

    f32 = lambda a: np.ascontiguousarray(np.asarray(a), dtype=np.float32)
    x = f32(inputs['x'])
    names = ['fc1_w', 'fc1_b', 'bn1_g', 'bn1_b', 'conv_w', 'conv_b',
             'bnc_g', 'bnc_b', 'fc2_w', 'fc2_b', 'bn2_g', 'bn2_b']
    w = {n: f32(inputs[n]) for n in names}

    in_maps = []
    for d in range(8):
        b, qoff = d // 4, (d % 4) * NQ
        m = dict(w)
        m['x_own'] = np.ascontiguousarray(np.roll(x[b], -qoff, axis=0))
        m['x_oth'] = np.ascontiguousarray(x[1 - b])
        in_maps.append(m)

    r1 = bass_utils.run_bass_kernel_spmd(nc1, in_maps, core_ids=list(range(8)))
    _CACHE['last_res'] = r1

    in_maps2 = []
    for d in range(8):
        rr = r1.results[d]
        idx = rr['idx_out'].astype(np.int64)          # [NQ, 9]
        maxn = rr['h_tok_out'][idx].max(axis=1)       # [NQ, C]
        m2 = {n: w[n] for n in ['conv_w', 'conv_b', 'bnc_g', 'bnc_b',
                                'fc2_w', 'fc2_b', 'bn2_g', 'bn2_b']}
        m2['h_fm'] = rr['h_fm_out']
        m2['maxn_fm'] = np.ascontiguousarray(maxn.T)
        m2['x_res'] = np.ascontiguousarray(in_maps[d]['x_own'][:NQ])
        in_maps2.append(m2)

    r2 = bass_utils.run_bass_kernel_spmd(nc2, in_maps2, core_ids=list(range(8)))
    _CACHE['last_res2'] = r2

    out = np.empty((B, N, C), np.float32)
    for d in range(8):
        b, qoff = d // 4, (d % 4) * NQ
        out[b, qoff:qoff + NQ] = r2.results[d]['y']
    return out

